# revision 32
# baseline (speedup 1.0000x reference)
"""SAM-style global attention (1,64,64,768), 12 heads, on 8 TRN2 NeuronCores.

Sharding: 24 units of (head, query-half-of-2048). Core c owns units
[3c, 3c+3) = 1.5 heads of queries spanning exactly 2 heads. Each core emits a
partial projected output outT (768, 4096); host sums the 8 partials, adds
proj_b + projected v-bias (a constant vector), transposes.

SPMD trick (as before): odd cores get their TOKEN ORDER half-swapped on the
host so one graph serves all cores; local head 0 = the fully-owned head.

Numerics (rel-err budget 2e-2; proj_b shields attention-level errors ~4x):
  - qkv projections: fp8e4 DoubleRow with hi/lo residual splitting of x
    (and of w for V), recovering ~bf16 accuracy at 2x bf16 PE throughput.
  - S^T: ONE fp8 DoubleRow matmul per (kb, qt):
      pair0: [k*4 ; onehot_kw*8] x [ (scale*q+qb)*32 ; relw*16 ]
      pair1: [onehot_kh*8 ; 0]   x [ relh*16 ; 0 ]
    PSUM holds 128*S; rel-pos biases folded in.
  - exp: per key-block on ScalarE (exact exp, scale=1/128, bias=ln4, out
    fp8e4) or DVE (one-instr Schraudolph: int8 = S*(log2e*8/128)+B, bitcast
    fp8e4, ~3% rms, mean-centered). GPSIMD/Pool cannot touch PSUM.
  - ctx^T: fp8 DoubleRow over kb pairs; V' = [v*2 ; ones*2]; row 64 = denom.
"""

import numpy as np
import ml_dtypes

NH, HD, Hh, Ww, DIM = 12, 64, 64, 64, 768
HW = Hh * Ww  # 4096
SCALE = HD ** -0.5
NCORES = 8
BF16 = ml_dtypes.bfloat16
F8 = ml_dtypes.float8_e4m3

LOG2E = 1.4426950408889634
AQ, AK, AREL, AOH, AE, AVs, SSC = 32.0, 4.0, 16.0, 8.0, 4.0, 2.0, 128.0
AWK, AWQ, AWV, AX = 16.0, 64.0, 16.0, 1.0
SCH_A = LOG2E * 8.0 / SSC
SCH_B = 56.0 + 8.0 * 2.0 + 0.54  # 2 = log2(AE); +0.54 centers trunc-convert


# exp engine per [128,1024] key-block slab (32 per qt): A=ScalarE exact exp
# (996 ns), D=DVE schraudolph (1192 ns). DVE also carries recip/norm/proj
# copies, so it gets fewer slabs than pure rate-balance.
def _mk_exp_pattern(n=32, rates=(('A', 996.0), ('D', 1470.0))):
    acc = {e: 0.0 for e, _ in rates}
    rate = dict(rates)
    pat = []
    for _ in range(n):
        e = min(acc, key=lambda k: acc[k] + rate[k])
        pat.append(e)
        acc[e] += rate[e]
    return pat


EXP_PAT = _mk_exp_pattern()

LAST_EXEC_NS = None
_PROGRAM = None


def _core_units(c):
    units = [(u // 2, u % 2) for u in range(3 * c, 3 * c + 3)]
    heads = sorted({h for h, _ in units})
    return units, heads


def _hi_lo_f8(a):
    hi = np.asarray(a, dtype=F8)
    lo = (np.asarray(a, np.float32) - hi.astype(np.float32)).astype(F8)
    return hi, lo


def _prep_core_inputs(c, x, qkv_w, qkv_b, proj_w, rel_pos_h, rel_pos_w):
    f32 = np.float32
    units, heads = _core_units(c)
    swapped = (c % 2 == 1)
    if swapped:
        h0, h1 = heads[1], heads[0]
    else:
        h0, h1 = heads[0], heads[1]

    xflat = x.reshape(HW, DIM).astype(f32)
    if swapped:
        xflat = np.concatenate([xflat[2048:], xflat[:2048]], axis=0)
    xT = np.ascontiguousarray(xflat.T) * AX        # (768, 4096) f32
    xhi, xlo = _hi_lo_f8(xT)

    def wslice(base, h):
        return qkv_w[base + h * 64: base + h * 64 + 64, :].astype(f32)

    def pack_w(wa, wb, alpha):
        # (64,768) x2 -> (768, 128) scaled, in hi/lo fp8, laid out so that
        # [128, 6*128] slices give chunk-pair DoubleRow APs
        wt = np.concatenate([wa.T, wb.T], axis=1) * alpha  # (768, 128)
        hi, lo = _hi_lo_f8(wt)
        return (np.ascontiguousarray(hi.reshape(6, 128, 128)),
                np.ascontiguousarray(lo.reshape(6, 128, 128)))

    wkh, wkl = pack_w(wslice(768, h0), wslice(768, h1), AWK)
    wqh, wql = pack_w(SCALE * wslice(0, h0), SCALE * wslice(0, h1), AWQ)
    wvh, wvl = pack_w(wslice(1536, h0), wslice(1536, h1), AWV)
    # wall8[p, kind*768 + i*128 : +128]; kinds: khi,qhi,vhi,kl?,ql?,vlo
    wall8 = np.zeros((128, 6 * 768), dtype=F8)
    for i in range(6):
        for kind, w in enumerate((wkh, wqh, wvh, wkl, wql, wvl)):
            wall8[:, kind * 768 + i * 128: kind * 768 + i * 128 + 128] = w[i]

    qb = np.concatenate([
        qkv_b[h0 * 64: h0 * 64 + 64],
        qkv_b[h1 * 64: h1 * 64 + 64],
    ]).astype(f32).reshape(128, 1)
    ball = np.ascontiguousarray(SCALE * AQ * qb)  # (128, 1) f32

    a = np.arange(64)
    perm = ((a + 32) % 64) if swapped else a
    idx_h = perm[:, None] - perm[None, :] + 63     # (qh_new, kh_new)
    idx_w = a[:, None] - a[None, :] + 63           # (qw, kw)

    def gather(tab, idx):
        g = np.transpose(tab[idx], (2, 0, 1)).reshape(HD, HW) * 8.0
        return np.ascontiguousarray(
            np.concatenate([g, g], axis=0)).astype(BF16)  # (128, 4096)

    relh = gather(rel_pos_h, idx_h)
    relw = gather(rel_pos_w, idx_w)
    relpack = np.ascontiguousarray(
        np.concatenate([relh, relw], axis=1))  # (128, 2*HW) bf16

    k = np.arange(HW)
    ohkw8 = np.ascontiguousarray(
        (k[None, :] % 64 == a[:, None]) * AOH).astype(F8)   # (64, 4096)
    ohkhz8 = np.zeros((128, HW), dtype=F8)
    ohkhz8[0:64] = ((k[None, :] // 64 == a[:, None]) * AOH).astype(F8)

    # projT packed (128, 1536) bf16:
    #   cols 0:768  = half-0 stacked lhsT (rows 0:64 = h0/j0, 64:128 = h1/j1)
    #   cols 768:1536 = half-1 lhsT (rows 0:64 = h0/j0; rows 64:128 zero)
    pj0 = proj_w[:, h0 * 64: h0 * 64 + 64].T.astype(f32)  # (64, 768)
    pj1 = proj_w[:, h1 * 64: h1 * 64 + 64].T.astype(f32)
    projT = np.zeros((128, 1536), dtype=BF16)
    projT[0:64, 0:768] = pj0.astype(BF16)
    projT[64:128, 0:768] = pj1.astype(BF16)
    projT[0:64, 768:1536] = pj0.astype(BF16)

    return dict(xhi=np.ascontiguousarray(xhi), xlo=np.ascontiguousarray(xlo),
                wall8=wall8, ball=ball, relpack=relpack,
                ohkw8=ohkw8, ohkhz8=ohkhz8, projT=projT)


def _build_program():
    import concourse.bacc as bacc
    import concourse.tile as tile
    import concourse.mybir as mybir

    f32 = mybir.dt.float32
    bf16 = mybir.dt.bfloat16
    f8 = mybir.dt.float8e4
    i8 = mybir.dt.int8
    AF = mybir.ActivationFunctionType
    ALU = mybir.AluOpType
    DR = mybir.MatmulPerfMode.DoubleRow

    nc = bacc.Bacc("TRN2", target_bir_lowering=False, debug=False,
                   enable_asserts=False, num_devices=NCORES)

    def din(name, shape, dt=bf16):
        return nc.dram_tensor(name, list(shape), dt, kind="ExternalInput").ap()

    xhi_d = din("xhi", (DIM, HW), f8)
    xlo_d = din("xlo", (DIM, HW), f8)
    wall8_d = din("wall8", (128, 6 * 768), f8)
    ball_d = din("ball", (128, 1), f32)
    relpack_d = din("relpack", (128, 2 * HW))
    ohkw8_d = din("ohkw8", (64, HW), f8)
    ohkhz8_d = din("ohkhz8", (128, HW), f8)
    projT_d = din("projT", (128, 1536))
    outT_d = nc.dram_tensor("outT", [DIM, HW], bf16,
                            kind="ExternalOutput").ap()

    sched = [(0, 0), (0, 1), (1, 0)]  # canonical (local head j, q-half)

    with tile.TileContext(nc) as tc:
        with tc.tile_pool(name="persist", bufs=1) as P:
            # ---- DMA: x hi/lo split across the two HWDGE queues ----
            wall8_s = P.tile([128, 6 * 768], f8, name="wall8")
            nc.sync.dma_start(wall8_s[:, 0:2304], wall8_d[:, 0:2304])
            nc.scalar.dma_start(wall8_s[:, 2304:4608], wall8_d[:, 2304:4608])

            # w slices for DoubleRow chunk-pair APs: [128, 2, 128] views
            def wpair(kind, cp):
                base = kind * 768 + cp * 256
                return wall8_s[:, base: base + 256].rearrange(
                    "p (c m) -> p c m", c=2)

            xhi = P.tile([128, 6, HW], f8, name="xhi")
            xlo = P.tile([128, 6, HW], f8, name="xlo")
            xhi_r = xhi_d.rearrange("(c p) t -> c p t", p=128)
            xlo_r = xlo_d.rearrange("(c p) t -> c p t", p=128)
            for i in range(6):
                nc.sync.dma_start(xhi[:, i, :], xhi_r[i])
                nc.scalar.dma_start(xlo[:, i, :], xlo_r[i])

            ball_s = P.tile([128, 1], f32, name="ball")
            nc.sync.dma_start(ball_s[:, :], ball_d)
            qb_s = ball_s[:, 0:1]

            # fp8 pair tensors for the S DoubleRow matmul
            # ko8: kb-major [128, kb, pair*128] so the DoubleRow weights
            # slice per kb has a small pair stride (ISA limit on lhsT)
            ko8 = [P.tile([128, 32, 256], f8, name=f"ko8{j}")
                   for j in range(2)]
            qr8 = [P.tile([128, 2, HW], f8, name=f"qr8{j}") for j in range(2)]
            ohkw_r = ohkw8_d.rearrange("p (b m) -> p b m", m=128)
            ohkhz_r = ohkhz8_d.rearrange("p (b m) -> p b m", m=128)
            nc.sync.dma_start(ko8[0][64:128, :, 0:128], ohkw_r)
            nc.scalar.dma_start(ko8[1][0:64, :, 0:128], ohkw_r)
            nc.sync.dma_start(ko8[0][:, :, 128:256], ohkhz_r)
            nc.scalar.dma_start(ko8[1][:, :, 128:256], ohkhz_r)
            # zero the unused lower half of qr8 pair1 (multiplied by the
            # zero rows of ko8 pair1; keeps NaNs out). Pool is SBUF-only
            # but that is exactly what this is.
            nc.gpsimd.memset(qr8[0][64:128, 1, :], 0)
            nc.gpsimd.memset(qr8[1][64:128, 1, :], 0)

            relpack_s = P.tile([128, 2 * HW], bf16, name="relpack")
            nc.sync.dma_start(relpack_s[:, :], relpack_d)
            relh_s = relpack_s[:, 0:HW]
            relw_s = relpack_s[:, HW:2 * HW]

            projT_s = P.tile([128, 1536], bf16, name="projT")
            nc.scalar.dma_start(projT_s[:, :], projT_d)

            # V' pair tensors, one per head: [128, kb, 128] with cols
            # [v(64) | ones*AVs(1) | zeros(63)]; kb-pair slices are then
            # contiguous 128-wide DoubleRow weights (ISA requirement).
            vp8 = [P.tile([128, 32, 128], f8, name=f"vp8{j}")
                   for j in range(2)]
            for j in range(2):
                nc.gpsimd.memset(vp8[j][:, :, 64:65], AVs)
                nc.gpsimd.memset(vp8[j][:, :, 65:128], 0)

            # ctx outputs (bf16): half0 stacked [128, 2048], half1 [64, 2048]
            ctx0_sb = P.tile([128, 2048], bf16, name="ctx0")
            ctx1_sb = P.tile([64, 2048], bf16, name="ctx1")

            ones_s = P.tile([65, 64], bf16, name="ones")
            nc.vector.memset(ones_s[64:65, :], 1.0)
            expb_s = P.tile([128, 1], f32, name="expb")
            nc.gpsimd.memset(expb_s[:, :], float(np.log(AE)))

            # PE warm-up during the input-DMA wait (p-state ramp)
            warm_row = P.tile([65, 512], bf16, name="warmrow")
            nc.vector.memset(warm_row[64:65, :], 1.0)
            with tc.tile_pool(name="warm", bufs=1, space="PSUM") as WP:
                wt = WP.tile([64, 512], f32, name="warmt")
                for _w in range(16):
                    nc.tensor.matmul(wt[:, :], ones_s[64:65, :],
                                     warm_row[64:65, :],
                                     start=True, stop=True,
                                     tile_position=(64, 0))

            # ---------------- phase 1: projections ----------------
            # All psum->sbuf writes on ScalarE/DVE (GPSIMD can't read PSUM).
            with tc.tile_pool(name="p1", bufs=1, space="PSUM") as PP:
                # K then Q: hi-main + xlo-compensation DoubleRow passes
                for kind, wcol, wr_scale, name in ((0, 0, AK / AWK, "K"),
                                                   (1, 1, AQ / AWQ, "Q")):
                    pk = [PP.tile([128, 512], f32, name=f"p{name}{t}",
                                  tag=f"pqk{t}") for t in range(8)]
                    # full bilinear: whi.xhi + whi.xlo + wlo.xhi per pair
                    steps = [(wcol, 0, xhi), (wcol, 1, xhi), (wcol, 2, xhi),
                             (wcol, 0, xlo), (wcol, 1, xlo), (wcol, 2, xlo),
                             (wcol + 3, 0, xhi), (wcol + 3, 1, xhi),
                             (wcol + 3, 2, xhi)]
                    for n, (wk, cp, xs) in enumerate(steps):
                        for t in range(8):
                            ts = slice(t * 512, t * 512 + 512)
                            nc.tensor.matmul(
                                pk[t][:, :], wpair(wk, cp),
                                xs[:, 2 * cp: 2 * cp + 2, ts],
                                start=(n == 0), stop=(n == 8), perf_mode=DR)
                    for t in range(8):
                        ts = slice(t * 512, t * 512 + 512)
                        if name == "K":
                            dA = ko8[0][0:64, 4 * t: 4 * t + 4, 0:128]
                            dB = ko8[1][64:128, 4 * t: 4 * t + 4, 0:128]
                            sA = pk[t][0:64, :].rearrange(
                                "p (b m) -> p b m", m=128)
                            sB = pk[t][64:128, :].rearrange(
                                "p (b m) -> p b m", m=128)
                            if t % 2 == 0:
                                nc.scalar.activation(dA, sA,
                                                     AF.Copy, scale=wr_scale)
                                nc.vector.tensor_scalar(
                                    dB, sB, wr_scale, None, ALU.mult)
                            else:
                                nc.vector.tensor_scalar(
                                    dA, sA, wr_scale, None, ALU.mult)
                                nc.scalar.activation(dB, sB,
                                                     AF.Copy, scale=wr_scale)
                        else:
                            if t % 2 == 0:
                                nc.vector.tensor_scalar(
                                    qr8[0][0:64, 0, ts], pk[t][0:64, :],
                                    wr_scale, qb_s[0:64, :],
                                    ALU.mult, ALU.add)
                            else:
                                nc.scalar.activation(
                                    qr8[0][0:64, 0, ts], pk[t][0:64, :],
                                    AF.Identity, bias=qb_s[0:64, :],
                                    scale=wr_scale)
                            if t < 4:  # local head 1 only serves q-half 0
                                if t % 2 == 0:
                                    nc.scalar.activation(
                                        qr8[1][64:128, 0, ts],
                                        pk[t][64:128, :], AF.Identity,
                                        bias=qb_s[64:128, :], scale=wr_scale)
                                else:
                                    nc.vector.tensor_scalar(
                                        qr8[1][64:128, 0, ts],
                                        pk[t][64:128, :], wr_scale,
                                        qb_s[64:128, :], ALU.mult, ALU.add)

                # V: full hi/lo bilinear (v-error passes 1:1 into ctx):
                # whi.x_hi + whi.x_lo + wlo.x_hi per chunk-pair
                for b in range(32):
                    bs = slice(b * 128, b * 128 + 128)
                    pv = PP.tile([128, 128], f32, tag=f"pqk{b % 4}", name="pv")
                    steps = [(xhi, 2), (xlo, 2), (xhi, 5)]
                    n = 0
                    for cp in range(3):
                        for xs, wk in steps:
                            nc.tensor.matmul(
                                pv[:, :], xs[:, 2 * cp: 2 * cp + 2, bs],
                                wpair(wk, cp),
                                start=(n == 0), stop=(n == 8), perf_mode=DR)
                            n += 1
                    for j in range(2):
                        dst = vp8[j][:, b, 0:64]
                        src = pv[:, j * 64: j * 64 + 64]
                        if (b + j) % 2 == 0:
                            nc.vector.tensor_scalar(dst, src, AVs / AWV,
                                                    None, ALU.mult)
                        else:
                            nc.scalar.activation(dst, src, AF.Copy,
                                                 scale=AVs / AWV)

                # RelW^T into qr8 pair0 (strided q cols); psum = 32*relw_true
                for j in range(2):
                    rows = slice(0, 64) if j == 0 else slice(64, 128)
                    orows = slice(64, 128) if j == 0 else slice(0, 64)
                    nqh = 64 if j == 0 else 32
                    qp_r = qr8[j][:, 0, :].rearrange(
                        "p (qh qw) -> p qw qh", qw=64)
                    for g in range(8):
                        pr = PP.tile([128, 512], f32, tag=f"pqk{6 + g % 2}",
                                     name="prw")
                        for qi in range(8):
                            qw = g * 8 + qi
                            nc.tensor.matmul(
                                pr[orows, qi * nqh: qi * nqh + nqh],
                                relw_s[rows, qw * 64: qw * 64 + 64],
                                qp_r[rows, qw, 0:nqh],
                                start=True, stop=True)
                        src = pr[orows, 0:8 * nqh].rearrange(
                            "p (qw qh) -> p qw qh", qh=nqh)
                        dst = qp_r[orows, g * 8:(g + 1) * 8, 0:nqh]
                        if g % 2 == 0:
                            nc.vector.tensor_scalar(dst, src, 0.5, None,
                                                    ALU.mult)
                        else:
                            nc.scalar.activation(dst, src, AF.Copy, scale=0.5)

                # RelH^T[kh, q] per head into qr8 pair1 rows 0:64 only (the
                # lower half is zeroed; ko8 pair1 rows 64:128 are zero too)
                for j in range(2):
                    rows = slice(0, 64) if j == 0 else slice(64, 128)
                    for g in range(8 if j == 0 else 4):
                        pr = PP.tile([128, 512], f32, tag=f"pqk{4 + g % 2}",
                                     name="prh")
                        for qi in range(8):
                            qh = g * 8 + qi
                            qs = slice(qh * 64, qh * 64 + 64)
                            cs = slice(qi * 64, qi * 64 + 64)
                            nc.tensor.matmul(pr[0:64, cs], relh_s[rows, qs],
                                             qr8[j][rows, 0, qs],
                                             start=True, stop=True)
                        dst = qr8[j][0:64, 1, g * 512: g * 512 + 512]
                        if g % 2 == 0:
                            nc.scalar.activation(dst, pr[0:64, :], AF.Copy,
                                                 scale=0.5)
                        else:
                            nc.vector.tensor_scalar(dst, pr[0:64, :], 0.5,
                                                    None, ALU.mult)

            # ---------------- attention + proj interleave ----------------
            with tc.tile_pool(name="ps", bufs=3, space="PSUM") as PS, \
                 tc.tile_pool(name="pc", bufs=1, space="PSUM") as PC, \
                 tc.tile_pool(name="esb", bufs=4) as ES, \
                 tc.tile_pool(name="pos", bufs=4) as POS:

                def attention_qt(uidx, j, half, qt, interleave=()):
                    inter = list(interleave)
                    qtb = half * 2048 + qt * 1024
                    qsl = slice(qtb, qtb + 1024)
                    ctx_ps = PC.tile([128, 1024], f32, tag="ctx", name="ctxps")
                    pend = []
                    e8 = None
                    for kb in range(32):
                        if inter and kb % 5 == 4:
                            inter.pop(0)()
                        kbs = slice(kb * 128, kb * 128 + 128)
                        s_t = PS.tile([128, 1024], f32, tag="s", name="st")
                        ko_kb = ko8[j][:, kb, :].rearrange(
                            "p (c m) -> p c m", c=2)
                        for sh in range(2):
                            shsl = slice(qtb + sh * 512, qtb + sh * 512 + 512)
                            nc.tensor.matmul(s_t[:, sh * 512: sh * 512 + 512],
                                             ko_kb, qr8[j][:, :, shsl],
                                             start=True, stop=True,
                                             perf_mode=DR)
                        if kb % 2 == 0:
                            e8 = ES.tile([128, 2, 1024], f8, tag="e", bufs=6)
                        eslot = e8[:, kb % 2, :]
                        if EXP_PAT[kb] == 'A':
                            nc.scalar.activation(eslot, s_t[:, :], AF.Exp,
                                                 bias=expb_s[:, :],
                                                 scale=1.0 / SSC)
                        else:
                            nc.vector.tensor_scalar(eslot.bitcast(i8),
                                                    s_t[:, :], SCH_A, SCH_B,
                                                    ALU.mult, ALU.add)
                        if kb % 2 == 1:
                            pend.append((kb // 2, e8))
                        if len(pend) > 2:
                            p, pe8 = pend.pop(0)
                            for sh in range(2):
                                ssl = slice(sh * 512, sh * 512 + 512)
                                nc.tensor.matmul(
                                    ctx_ps[:, ssl],
                                    vp8[j][:, 2 * p: 2 * p + 2, :],
                                    pe8[:, :, ssl],
                                    start=(p == 0), stop=False, perf_mode=DR)
                    for n, (p, pe8) in enumerate(pend):
                        for sh in range(2):
                            ssl = slice(sh * 512, sh * 512 + 512)
                            nc.tensor.matmul(
                                ctx_ps[:, ssl],
                                vp8[j][:, 2 * p: 2 * p + 2, :],
                                pe8[:, :, ssl],
                                start=(p == 0), stop=(p == 15), perf_mode=DR)

                    # normalize: ctx[0:64] * broadcast(1/den); v-bias on host
                    den = ES.tile([65, 1024], bf16, tag="den")
                    with nc.allow_low_precision(
                            reason="softmax denom recip; common factor "
                            "per column, 2^-9 rel err"):
                        nc.vector.reciprocal(den[64:65, :],
                                             ctx_ps[64:65, :])
                    pb_t = ctx_ps[64:128, :]
                    for sh in range(2):
                        ssl = slice(sh * 512, sh * 512 + 512)
                        nc.tensor.matmul(pb_t[:, ssl], ones_s[64:65, :],
                                         den[64:65, ssl],
                                         start=True, stop=True,
                                         tile_position=(64, 64))
                    bc_s = ES.tile([64, 1024], f32, tag="bcs")
                    nc.scalar.activation(bc_s[:, 0:512], pb_t[:, 0:512],
                                         AF.Copy)
                    nc.vector.tensor_copy(bc_s[:, 512:1024],
                                          pb_t[:, 512:1024])
                    if uidx == 0:
                        cdst = ctx0_sb[0:64, qt * 1024: qt * 1024 + 1024]
                    elif uidx == 2:
                        cdst = ctx0_sb[64:128, qt * 1024: qt * 1024 + 1024]
                    else:
                        cdst = ctx1_sb[0:64, qt * 1024: qt * 1024 + 1024]
                    nc.vector.tensor_tensor(cdst, ctx_ps[0:64, :],
                                            bc_s[:, :], op=ALU.mult)

                def proj_ocb(half, qt, ocb):
                    # half0: stacked units (contraction 128); half1: unit 1
                    wcol = 0 if half == 0 else 768
                    rows = slice(0, 128) if half == 0 else slice(0, 64)
                    csb = ctx0_sb if half == 0 else ctx1_sb
                    po_t = PS.tile([128, 1024], f32, tag="s", name="pot")
                    for sh in range(2):
                        ssl = slice(sh * 512, sh * 512 + 512)
                        nc.tensor.matmul(
                            po_t[:, ssl],
                            projT_s[rows, wcol + ocb * 128:
                                    wcol + ocb * 128 + 128],
                            csb[rows, qt * 1024 + sh * 512:
                                qt * 1024 + sh * 512 + 512],
                            start=True, stop=True)
                    po_s = POS.tile([128, 1024], bf16, tag="pos")
                    if ocb % 2 == 0:
                        nc.scalar.activation(po_s[:, :], po_t[:, :], AF.Copy)
                    else:
                        nc.vector.tensor_copy(po_s[:, :], po_t[:, :])
                    nc.sync.dma_start(
                        outT_d[ocb * 128: ocb * 128 + 128,
                               half * 2048 + qt * 1024:
                               half * 2048 + qt * 1024 + 1024],
                        po_s[:, :])

                def proj_calls(half, qt):
                    from functools import partial
                    return [partial(proj_ocb, half, qt, ocb)
                            for ocb in range(6)]

                attention_qt(0, *sched[0], 0)
                attention_qt(0, *sched[0], 1)
                attention_qt(1, *sched[1], 0)
                attention_qt(1, *sched[1], 1, interleave=proj_calls(1, 0))
                attention_qt(2, *sched[2], 0, interleave=proj_calls(1, 1))
                attention_qt(2, *sched[2], 1, interleave=proj_calls(0, 0))
                for call in proj_calls(0, 1):
                    call()

    nc.compile()
    return nc


def kernel(x, qkv_w, qkv_b, proj_w, proj_b, rel_pos_h, rel_pos_w, num_heads):
    global LAST_EXEC_NS, _PROGRAM
    from concourse.bass_utils import run_bass_kernel_spmd

    x = np.asarray(x, dtype=np.float32)
    qkv_w = np.asarray(qkv_w, dtype=np.float32)
    qkv_b = np.asarray(qkv_b, dtype=np.float32)
    proj_w = np.asarray(proj_w, dtype=np.float32)
    proj_b = np.asarray(proj_b, dtype=np.float32)
    rel_pos_h = np.asarray(rel_pos_h, dtype=np.float32)
    rel_pos_w = np.asarray(rel_pos_w, dtype=np.float32)
    assert int(num_heads) == NH

    in_maps = [_prep_core_inputs(c, x, qkv_w, qkv_b, proj_w,
                                 rel_pos_h, rel_pos_w) for c in range(NCORES)]

    if _PROGRAM is None:
        _PROGRAM = _build_program()
    nc = _PROGRAM

    import os
    trace = os.environ.get("KERNEL_TRACE", "0") == "1"
    try:
        res = run_bass_kernel_spmd(nc, in_maps, core_ids=list(range(NCORES)),
                                   trace=trace)
    except ModuleNotFoundError:
        res = run_bass_kernel_spmd(nc, in_maps, core_ids=list(range(NCORES)),
                                   trace=False)
    LAST_EXEC_NS = res.exec_time_ns

    out = np.zeros((DIM, HW), dtype=np.float32)
    for c in range(NCORES):
        o = np.asarray(res.results[c]["outT"], dtype=np.float32)
        if c % 2 == 1:  # un-swap token halves
            o = np.concatenate([o[:, 2048:], o[:, :2048]], axis=1)
        out += o
    # bias: proj_b plus the projected v-bias (constant across tokens)
    vb_full = qkv_b[1536:2304].astype(np.float32)
    bias = proj_b + proj_w.astype(np.float32) @ vb_full
    out = out.T + bias[None, :]
    return out.reshape(1, Hh, Ww, DIM).astype(np.float32)
